# revision 19
# baseline (speedup 1.0000x reference)
"""Trainium2 Bass kernel for nn_InvLocalPatOrientConvolution.

Computation:
  1. Host: synthesize the 160-channel 5x5x5 conv filter from
     weight/zeroweight/basis_functions/wigner indices, fold the so3 grid into
     the e>=12 channels (direct 108-channel "B" conv), quantize weights and
     input to fp8-e4m3 hi/lo pairs, and lay out per-core operands.
  2. Device (8 NeuronCores, SPMD):
     - conv A (120 ch = e<12 x l) + conv B-direct (108 ch = (el2,mln) for
       e>=12, grid pre-applied) as fp8 DoubleRow PE matmuls. Input rows are
       (i,j,d) combos (25 tap-pairs x 16 ch = 400 rows). Per output tile:
       term1 = [w_hi|w_hi] . [x_hi|x_lo]  (exact in x; 15 (k,group) matmuls
       + 1 z-preshifted leftover-pair matmul), term2 = w_lo . x_hi with
       256-row k-tile packing (10 matmuls). The dropped w_lo.x_lo term is
       O(eps^2). Global power-of-2 weight scales are undone in the fp16
       grid / moment lhsTs.
     - grid stage mt0..2 (324 outs) from the A channels, 3 fp16 matmuls.
     - relu (scalar) + square (vector) into (108,4,432) fp16 mega-tiles;
       moment stage = 2x4 chained fp16 matmuls (num/den) in one PSUM bank.
     - finalize num/(den+eps)+bias on DVE, one chunk behind the grid stage.
     Sharding: batch (2) x output-X-slabs (4) -> 8 cores.
  3. Host: gather per-core slabs into the full (2,16,36,36,36) output.
"""

import os
import sys

for _p in ("/root/.axon_site/_ro/trn_rl_repo", "/opt/trn_rl_repo"):
    if os.path.isdir(_p) and _p not in sys.path:
        sys.path.insert(0, _p)

import numpy as np
import ml_dtypes

import concourse.mybir as mybir
from concourse import bacc
from concourse.tile import TileContext
from concourse.bass_utils import run_bass_kernel_spmd

# Problem constants (hardcoded per harness contract)
ORDER = 2
KS = 5            # conv kernel size
CIN = 16
COUT = 16
EPS = 1e-16
S = 10            # wigner rows
B = 2
D_IN = 40         # input spatial
D_OUT = 36        # output spatial
SLAB = 9          # output X planes per core (36/4)
SLAB_IN = SLAB + KS - 1   # 13 input X planes per core
NCORES = 8
YB = 12           # y-block per chunk
NCH = YB * D_OUT  # 432 columns per matmul chunk

F16 = mybir.dt.float16
F32 = mybir.dt.float32
F8 = mybir.dt.float8e4
E4M3 = ml_dtypes.float8_e4m3
DR = mybir.MatmulPerfMode.DoubleRow

_prog_cache = {}


def _build_program(inv_sB=1.0, inv_swnd=1.0):
    """Build the SPMD device program (identical on all 8 cores)."""
    nc = bacc.Bacc("TRN2")

    rhl_ds = [nc.dram_tensor(f"rhl{g}", [128, SLAB, 2, D_IN, D_OUT], F8,
                             kind="ExternalInput") for g in range(3)]
    rl_d = nc.dram_tensor("rl", [80, SLAB, 2, D_OUT, D_OUT], F8,
                          kind="ExternalInput")
    rh2_ds = [nc.dram_tensor(f"rh2{t}", [128, SLAB, 2, D_IN, D_OUT], F8,
                             kind="ExternalInput") for t in range(2)]
    wahi_d = nc.dram_tensor("wahi", [128, KS, 3, 2, 128], F8,
                            kind="ExternalInput")
    wbhi_d = nc.dram_tensor("wbhi", [128, KS, 3, 2, 112], F8,
                            kind="ExternalInput")
    wlahi_d = nc.dram_tensor("wlahi", [80, 2, 128], F8, kind="ExternalInput")
    wlbhi_d = nc.dram_tensor("wlbhi", [80, 2, 112], F8, kind="ExternalInput")
    walo_d = nc.dram_tensor("walo", [128, KS, 2, 2, 128], F8,
                            kind="ExternalInput")
    wblo_d = nc.dram_tensor("wblo", [128, KS, 2, 2, 112], F8,
                            kind="ExternalInput")
    gat_d = nc.dram_tensor("gat", [120, 3, 108], F16, kind="ExternalInput")
    wnd_d = nc.dram_tensor("wnd", [108, 4, 16], F16, kind="ExternalInput")
    wnd8_d = nc.dram_tensor("wnd8", [108, 2, 2, 16], F8, kind="ExternalInput")
    bias_d = nc.dram_tensor("bias", [16, 1], F32, kind="ExternalInput")
    y_d = nc.dram_tensor("y", [16, SLAB, D_OUT, D_OUT], F32,
                         kind="ExternalOutput")

    chunks = [(xr, cy * YB, YB) for xr in range(SLAB) for cy in range(3)]
    chunks = chunks[:-1] + [(SLAB - 1, 24, 4), (SLAB - 1, 28, 4),
                            (SLAB - 1, 32, 4)]

    with TileContext(nc) as tc:
        with tc.tile_pool(name="const", bufs=1) as cpool, \
             tc.tile_pool(name="ca", bufs=2) as capool, \
             tc.tile_pool(name="rr", bufs=2) as rpool, \
             tc.tile_pool(name="fin", bufs=2) as wpool, \
             tc.tile_pool(name="a_ps", bufs=1, space="PSUM") as a_pool, \
             tc.tile_pool(name="b_ps", bufs=1, space="PSUM") as b_pool, \
             tc.tile_pool(name="g_ps", bufs=1, space="PSUM") as g_pool, \
             tc.tile_pool(name="nd_ps", bufs=2, space="PSUM") as nd_pool:

            # ---- resident tiles. DMA order: chunk-0 deps first.
            rhls = [cpool.tile([128, SLAB, 2, D_IN, D_OUT], F8, tag=f"rhl{g}",
                               name=f"rhl{g}") for g in range(3)]
            rlt = cpool.tile([80, SLAB, 2, D_OUT, D_OUT], F8, tag="rlt")
            rh2s = [cpool.tile([128, SLAB, 2, D_IN, D_OUT], F8, tag=f"rh2{t}",
                               name=f"rh2{t}") for t in range(2)]
            wahit = cpool.tile([128, KS, 3, 2, 128], F8, tag="wahit")
            wbhit = cpool.tile([128, KS, 3, 2, 112], F8, tag="wbhit")
            wlahit = cpool.tile([80, 2, 128], F8, tag="wlahit")
            wlbhit = cpool.tile([80, 2, 112], F8, tag="wlbhit")
            walot = cpool.tile([128, KS, 2, 2, 128], F8, tag="walot")
            wblot = cpool.tile([128, KS, 2, 2, 112], F8, tag="wblot")
            gatt = cpool.tile([120, 3, 108], F16, tag="gatt")
            wndt = cpool.tile([108, 4, 16], F16, tag="wndt")
            wnd8t = cpool.tile([108, 2, 2, 16], F8, tag="wnd8t")
            biast = cpool.tile([16, 1], F32, tag="biast")

            # chunk-0 input slices + conv weights first
            for g in range(3):
                nc.sync.dma_start(out=rhls[g][:, 0], in_=rhl_ds[g][:, 0])
            nc.sync.dma_start(out=rlt[:, 0], in_=rl_d[:, 0])
            for t in range(2):
                nc.sync.dma_start(out=rh2s[t][:, 0], in_=rh2_ds[t][:, 0])
            nc.gpsimd.dma_start(out=wahit[:], in_=wahi_d[:])
            nc.gpsimd.dma_start(out=wlahit[:], in_=wlahi_d[:])
            nc.gpsimd.dma_start(out=walot[:], in_=walo_d[:])
            nc.gpsimd.dma_start(out=wbhit[:], in_=wbhi_d[:])
            nc.gpsimd.dma_start(out=wlbhit[:], in_=wlbhi_d[:])
            nc.gpsimd.dma_start(out=wblot[:], in_=wblo_d[:])
            nc.gpsimd.dma_start(out=gatt[:], in_=gat_d[:])
            nc.gpsimd.dma_start(out=wndt[:], in_=wnd_d[:])
            nc.gpsimd.dma_start(out=wnd8t[:], in_=wnd8_d[:])
            nc.gpsimd.dma_start(out=biast[:], in_=bias_d[:])
            for xr in range(1, SLAB):
                for g in range(3):
                    nc.sync.dma_start(out=rhls[g][:, xr], in_=rhl_ds[g][:, xr])
                nc.sync.dma_start(out=rlt[:, xr], in_=rl_d[:, xr])
                for t in range(2):
                    nc.sync.dma_start(out=rh2s[t][:, xr], in_=rh2_ds[t][:, xr])

            pending = None
            for (xr, y0, yb) in chunks:
                n = yb * D_OUT

                # ---- conv A (120 ch) and conv B-direct (108 ch) ----
                aps = a_pool.tile([120, NCH], F32, tag="aps")
                bps = b_pool.tile([108, NCH], F32, tag="bps")
                for out_ps, m, whi, wlhi, wlo in (
                        (aps, 120, wahit, wlahit, walot),
                        (bps, 108, wbhit, wlbhit, wblot)):
                    t = 0
                    for j in range(KS):
                        for g in range(3):
                            rhs = rhls[g][:, xr, 0:2, y0 + j:y0 + j + yb, :]
                            nc.tensor.matmul(out_ps[:, 0:n],
                                             whi[:, j, g, :, 0:m],
                                             rhs, start=(t == 0), stop=False,
                                             perf_mode=DR)
                            t += 1
                        for t2 in range(2):
                            rhs = rh2s[t2][:, xr, 0:2,
                                           y0 + j:y0 + j + yb, :]
                            nc.tensor.matmul(out_ps[:, 0:n],
                                             wlo[:, j, t2, :, 0:m],
                                             rhs, start=False, stop=False,
                                             perf_mode=DR)
                    nc.tensor.matmul(out_ps[:, 0:n], wlhi[:, :, 0:m],
                                     rlt[:, xr, 0:2, y0:y0 + yb, :],
                                     start=False, stop=True, perf_mode=DR)

                # conv-A psum -> SBUF fp16 for the grid stage
                ca = capool.tile([120, NCH], F16, tag="ca")
                nc.scalar.copy(ca[:, 0:n], aps[:, 0:n])

                # previous chunk's moment/finalize work: emitted here so the
                # PE never waits on the relu/square of the current chunk
                if pending is not None:
                    _emit_moments(nc, wndt, wnd8t, biast, wpool, nd_pool,
                                  y_d, pending)

                # ---- grid stage mt0..2 (3 matmuls, one PSUM bank each) ----
                gps = g_pool.tile([108, 3, 512], F32, tag="gps")
                for mt in range(3):
                    nc.tensor.matmul(gps[:, mt, 0:n], gatt[:, mt, :],
                                     ca[:, 0:n], start=True, stop=True)

                # ---- relu (scalar, fp16 + fp8 copy) + square (vector) ----
                rrel = rpool.tile([108, 4, NCH], F16, tag="rrel")
                nc.scalar.activation(rrel[:, 0:3, 0:n], gps[:, :, 0:n],
                                     mybir.ActivationFunctionType.Relu)
                nc.scalar.activation(rrel[:, 3, 0:n], bps[:, 0:n],
                                     mybir.ActivationFunctionType.Relu,
                                     scale=inv_sB)
                r2 = rpool.tile([108, 4, NCH], F16, tag="r2")
                nc.vector.tensor_mul(r2[:, :, 0:n], rrel[:, :, 0:n],
                                     rrel[:, :, 0:n])
                pending = (rrel, r2, xr, y0, yb, n)

            if pending is not None:
                _emit_moments(nc, wndt, wnd8t, biast, wpool, nd_pool, y_d,
                              pending)

    nc.finalize()
    return nc


def _emit_moments(nc, wndt, wnd8t, biast, wpool, nd_pool, y_d, st):
    """Moment matmuls (num: 4 chained fp16 on r2; den: 2 chained fp8-DR on
    rrel8, packed in one PSUM bank at column positions 0/64) + finalize +
    store. The den act-copy unfolds the fp8 weight scale S_WND."""
    rrel, r2, xr, y0, yb, n = st
    nd = nd_pool.tile([128, NCH], F32, tag="nd")
    for mt in range(4):
        nc.tensor.matmul(nd[0:16, 0:n], wndt[:, mt, :], r2[:, mt, 0:n],
                         start=(mt == 0), stop=(mt == 3),
                         tile_position=(0, 0))
        nc.tensor.matmul(nd[64:80, 0:n], wndt[:, mt, :], rrel[:, mt, 0:n],
                         start=(mt == 0), stop=(mt == 3),
                         tile_position=(0, 64))
    den_sb = wpool.tile([16, NCH], F32, tag="den_sb")
    nc.scalar.activation(den_sb[:, 0:n], nd[64:80, 0:n],
                         mybir.ActivationFunctionType.Copy, bias=EPS)
    recip = wpool.tile([16, NCH], F32, tag="recip")
    nc.vector.reciprocal(recip[:, 0:n], den_sb[:, 0:n])
    out_sb = wpool.tile([16, NCH], F32, tag="out_sb")
    nc.vector.tensor_mul(out_sb[:, 0:n], nd[0:16, 0:n], recip[:, 0:n])
    nc.vector.tensor_scalar_add(out_sb[:, 0:n], out_sb[:, 0:n],
                                biast[:, 0:1])
    nc.sync.dma_start(out=y_d[:, xr, y0:y0 + yb, :],
                      in_=out_sb[:, 0:n].rearrange("p (a b) -> p a b", a=yb))


def _synthesize_filter(weight, zeroweight, basis_functions, wig_w, wig_b):
    """Replicate the reference's kernel synthesis in fp32 numpy.

    Returns kern6[l, e, d, i, j, k] of shape (10, 16, 16, 5, 5, 5) where
    (e,l) indexes the 160 conv output channels and (d,i,j,k) the
    contraction."""
    zero_ext = np.concatenate(
        [zeroweight[None, None],
         np.zeros((ORDER ** 2 - 1, 1, CIN, COUT), weight.dtype)], axis=0)
    wfull = np.concatenate([zero_ext, weight], axis=1)       # (4, 10, 16, 16)
    wg = wfull[wig_w]                                        # (10, 10, 16, 16)
    bg = basis_functions[wig_b]                              # (10, 10, 5, 5, 5)
    kern6 = np.einsum("lred,lrijk->ledijk", wg, bg)          # (10,16,16,5,5,5)
    return np.ascontiguousarray(kern6.astype(np.float32))


def _q8_pair(a):
    hi = a.astype(E4M3)
    lo = (a - hi.astype(np.float32)).astype(E4M3)
    return hi, lo


def _pow2_scale(absmax, target=64.0):
    if absmax <= 0:
        return 1.0
    return 2.0 ** np.floor(np.log2(target / absmax))


def _host_prep(x, weight, zeroweight, bias, so3basisgrid, w_i,
               basis_functions, wig_w, wig_b):
    kern6 = _synthesize_filter(weight, zeroweight, basis_functions,
                               wig_w, wig_b)
    # Wf[pair(i,j), d, k, out(e*10+l)]
    Wf = np.ascontiguousarray(
        kern6.transpose(3, 4, 2, 5, 1, 0).reshape(25, 16, KS, 160))

    g2 = np.asarray(so3basisgrid, np.float32).reshape(27, S)  # [mln, l]
    w_flat = np.asarray(w_i, np.float32)[(np.arange(27) // 3) % 3]

    # B channels (e>=12): fold grid -> out (el2*27+mln)
    WfB = Wf[:, :, :, 120:].reshape(25, 16, KS, 4, S)        # (..., el2, l)
    kern2B = np.einsum("pdkel,ml->pdkem", WfB, g2).reshape(25, 16, KS, 108)
    WfA = Wf[:, :, :, 0:120]

    sA = _pow2_scale(np.abs(WfA).max())
    sB = _pow2_scale(np.abs(kern2B).max())
    WfA_hi, WfA_lo = _q8_pair(WfA * sA)          # (25, 16, KS, 120) e4m3
    kB_hi, kB_lo = _q8_pair(kern2B * sB)

    # conv lhsT tiles. Row space: rho = pair2*16+d with pair2 = i*5+k; the
    # j-tap picks the weight slice and the rhs y-offset. M padded to mp for
    # the 16B dual-fp8 pair-stride rule.
    # W2[pair2, d, j, out] = Wf[i*5+j, d, k, out]
    def reindex(w):
        m = w.shape[-1]
        w5 = w.reshape(KS, KS, 16, KS, m)                # (i, j, d, k, m)
        return np.ascontiguousarray(
            w5.transpose(0, 3, 2, 1, 4).reshape(25, 16, KS, m))

    def pack_hi(w_hi, m, mp):
        w_hi = reindex(w_hi)
        out = np.zeros((128, KS, 3, 2, mp), E4M3)
        for pair in range(24):
            g, p0 = divmod(pair * 16, 128)
            out[p0:p0 + 16, :, g, 0, 0:m] = w_hi[pair]   # (16, KS, m)
            out[p0:p0 + 16, :, g, 1, 0:m] = w_hi[pair]
        return out

    def pack_lo(w_lo, m, mp):
        # term2 k-tile pairs: t=0 -> rows (0..127 | 128..255),
        # t=1 -> rows (256..383 | 384..399 zero-padded)
        w_lo = reindex(w_lo)
        out = np.zeros((128, KS, 2, 2, mp), E4M3)
        for pair in range(25):
            rho0 = pair * 16
            t, rem = divmod(rho0, 256)
            gg, p0 = divmod(rem, 128)
            out[p0:p0 + 16, :, t, gg, 0:m] = w_lo[pair]
        return out

    def pack_leftover_hi(w_hi, m, mp):
        # leftover pair2 = (i=4, k=4); rows (j, d)
        w_hi = reindex(w_hi)
        out = np.zeros((80, 2, mp), E4M3)
        for j in range(KS):
            out[16 * j:16 * j + 16, 0, 0:m] = w_hi[24, :, j, :]
            out[16 * j:16 * j + 16, 1, 0:m] = w_hi[24, :, j, :]
        return out

    wahi = pack_hi(WfA_hi, 120, 128)
    wbhi = pack_hi(kB_hi, 108, 112)
    walo = pack_lo(WfA_lo, 120, 128)
    wblo = pack_lo(kB_lo, 108, 112)
    wlahi = pack_leftover_hi(WfA_hi, 120, 128)
    wlbhi = pack_leftover_hi(kB_hi, 108, 112)

    # grid lhsT for mt0..2: rows (e*10+l, e<12); undo sA here
    gat = np.zeros((120, 3, 108), np.float32)
    for mt in range(3):
        for el2 in range(4):
            e = 4 * mt + el2
            for l in range(S):
                gat[e * S + l, mt, el2 * 27:(el2 + 1) * 27] = g2[:, l]
    gat = (gat / sA).astype(np.float16)

    # moment lhsTs: rows (el2*27+mln) -> col e = 4*mt+el2.
    # mt3 rows see sB-scaled relu values: undo with 1/sB^2 (num), 1/sB (den)
    wnd = np.zeros((108, 4, 16), np.float32)
    for mt in range(4):
        for el2 in range(4):
            e = 4 * mt + el2
            wnd[el2 * 27:(el2 + 1) * 27, mt, e] = w_flat
    wnd = wnd.astype(np.float16)

    # fp8 den lhsT: brute-force a scale minimizing e4m3 rel err of w_i
    wvals = np.asarray(w_i, np.float32)
    best, s_wnd = 1e9, 64.0
    for s in np.linspace(32.0, 128.0, 4096):
        q = (wvals * s).astype(E4M3).astype(np.float32)
        e = np.abs(q / (wvals * s) - 1).max()
        if e < best:
            best, s_wnd = e, float(s)
    wnd8 = np.zeros((108, 2, 2, 16), E4M3)
    for mt in range(4):
        p, h = divmod(mt, 2)
        for el2 in range(4):
            e = 4 * mt + el2
            wnd8[el2 * 27:(el2 + 1) * 27, p, h, e] = (
                (w_flat * s_wnd).astype(E4M3))

    bias_arr = np.asarray(bias, np.float32).reshape(16, 1)

    # input hi/lo quantization (global, then per-core packing)
    x32 = np.asarray(x, np.float32)
    x_hi8 = x32.astype(E4M3)
    x_lo8 = (x32 - x_hi8.astype(np.float32)).astype(E4M3)

    in_maps = []
    for c in range(NCORES):
        b, q = divmod(c, 4)
        sl = slice(q * SLAB, q * SLAB + SLAB_IN)
        shi = x_hi8[b, :, sl]                    # (16, 13, 40, 40) e4m3
        slo = x_lo8[b, :, sl]
        rhl = [np.zeros((128, SLAB, 2, D_IN, D_OUT), E4M3) for _ in range(3)]
        rh2 = [np.zeros((128, SLAB, 2, D_IN, D_OUT), E4M3) for _ in range(2)]
        for pair in range(25):
            i, k = divmod(pair, KS)
            hi_blk = shi[:, i:i + SLAB, :, k:k + D_OUT]      # (16,9,40,36)
            hi_blk = hi_blk.transpose(0, 1, 2, 3)
            if pair < 24:
                g, p0 = divmod(pair * 16, 128)
                lo_blk = slo[:, i:i + SLAB, :, k:k + D_OUT]
                rhl[g][p0:p0 + 16, :, 0] = hi_blk
                rhl[g][p0:p0 + 16, :, 1] = lo_blk
            t, rem = divmod(pair * 16, 256)
            gg, p0 = divmod(rem, 128)
            rh2[t][p0:p0 + 16, :, gg] = hi_blk
        rl = np.empty((80, SLAB, 2, D_OUT, D_OUT), E4M3)
        for j in range(KS):
            rl[16 * j:16 * j + 16, :, 0] = shi[:, 4:4 + SLAB, j:j + D_OUT,
                                               4:4 + D_OUT]
            rl[16 * j:16 * j + 16, :, 1] = slo[:, 4:4 + SLAB, j:j + D_OUT,
                                               4:4 + D_OUT]
        in_maps.append({
            "rhl0": rhl[0], "rhl1": rhl[1], "rhl2": rhl[2], "rl": rl,
            "rh20": rh2[0], "rh21": rh2[1],
            "wahi": wahi, "wbhi": wbhi, "wlahi": wlahi, "wlbhi": wlbhi,
            "walo": walo, "wblo": wblo,
            "gat": np.ascontiguousarray(gat),
            "wnd": np.ascontiguousarray(wnd),
            "wnd8": np.ascontiguousarray(wnd8),
            "bias": bias_arr,
        })
    return in_maps, sB, s_wnd


def _run(inputs, trace=False, **run_kwargs):
    inputs = {k: np.asarray(v) for k, v in inputs.items()}
    in_maps, sB, s_wnd = _host_prep(**inputs)
    if _prog_cache.get("key") != (float(sB), float(s_wnd)):
        _prog_cache["nc"] = _build_program(1.0 / sB, 1.0 / s_wnd)
        _prog_cache["key"] = (float(sB), float(s_wnd))
    nc = _prog_cache["nc"]
    try:
        res = run_bass_kernel_spmd(nc, in_maps, core_ids=list(range(NCORES)),
                                   trace=trace, **run_kwargs)
    except ModuleNotFoundError as e:
        if "axon_hooks" not in str(e):
            raise
        os.environ["BASS_NEVER_TRACE"] = "1"
        res = run_bass_kernel_spmd(nc, in_maps, core_ids=list(range(NCORES)),
                                   trace=False, **run_kwargs)
    out = np.empty((B, COUT, D_OUT, D_OUT, D_OUT), np.float32)
    for c in range(NCORES):
        b, q = divmod(c, 4)
        out[b, :, q * SLAB:(q + 1) * SLAB] = res.results[c]["y"]
    return out, res


def kernel(**inputs):
    out, _ = _run(inputs)
    return out


# revision 32
# speedup vs baseline: 1.0701x; 1.0701x over previous
"""Trainium2 Bass kernel for nn_InvLocalPatOrientConvolution.

Computation:
  1. Host: synthesize the 160-channel 5x5x5 conv filter from
     weight/zeroweight/basis_functions/wigner indices, fold the so3 grid into
     the e>=12 channels (direct 108-channel "B" conv), quantize weights and
     input to fp8-e4m3 hi/lo pairs, and lay out per-core operands.
  2. Device (8 NeuronCores, SPMD):
     - conv A (120 ch = e<12 x l) + conv B-direct (108 ch = (el2,mln) for
       e>=12, grid pre-applied) as fp8 DoubleRow PE matmuls. Input rows are
       (i,j,d) combos (25 tap-pairs x 16 ch = 400 rows). Per output tile:
       term1 = [w_hi|w_hi] . [x_hi|x_lo]  (exact in x; 15 (k,group) matmuls
       + 1 z-preshifted leftover-pair matmul), term2 = w_lo . x_hi with
       256-row k-tile packing (10 matmuls). The dropped w_lo.x_lo term is
       O(eps^2). Global power-of-2 weight scales are undone in the fp16
       grid / moment lhsTs.
     - grid stage mt0..2 (324 outs) from the A channels, 3 fp16 matmuls.
     - relu (scalar) + square (vector) into (108,4,432) fp16 mega-tiles;
       moment stage = 2x4 chained fp16 matmuls (num/den) in one PSUM bank.
     - finalize num/(den+eps)+bias on DVE, one chunk behind the grid stage.
     Sharding: batch (2) x output-X-slabs (4) -> 8 cores.
  3. Host: gather per-core slabs into the full (2,16,36,36,36) output.
"""

import os
import sys

for _p in ("/root/.axon_site/_ro/trn_rl_repo", "/opt/trn_rl_repo"):
    if os.path.isdir(_p) and _p not in sys.path:
        sys.path.insert(0, _p)

import numpy as np
import ml_dtypes

import concourse.mybir as mybir
from concourse import bacc
from concourse.tile import TileContext
from concourse.bass_utils import run_bass_kernel_spmd

# Problem constants (hardcoded per harness contract)
ORDER = 2
KS = 5            # conv kernel size
CIN = 16
COUT = 16
EPS = 1e-16
S = 10            # wigner rows
B = 2
D_IN = 40         # input spatial
D_OUT = 36        # output spatial
SLAB = 9          # output X planes per core (36/4)
SLAB_IN = SLAB + KS - 1   # 13 input X planes per core
NCORES = 8
YB = 12           # y-block per chunk
NCH = YB * D_OUT  # 432 columns per matmul chunk

F16 = mybir.dt.float16
F32 = mybir.dt.float32
F8 = mybir.dt.float8e4
E4M3 = ml_dtypes.float8_e4m3
DR = mybir.MatmulPerfMode.DoubleRow

_prog_cache = {}


def _build_program(inv_sB=1.0, zero_bias=False):
    """Build the SPMD device program (identical on all 8 cores)."""
    nc = bacc.Bacc("TRN2")

    rall_d = nc.dram_tensor("rall", [128, 5, SLAB, 2, D_IN, D_OUT], F8,
                            kind="ExternalInput")
    rl_d = nc.dram_tensor("rl", [80, SLAB, 2, D_OUT, D_OUT], F8,
                          kind="ExternalInput")
    wahi_d = nc.dram_tensor("wahi", [128, KS, 3, 2, 128], F8,
                            kind="ExternalInput")
    wbhi_d = nc.dram_tensor("wbhi", [128, KS, 3, 2, 112], F8,
                            kind="ExternalInput")
    wlahi_d = nc.dram_tensor("wlahi", [80, 2, 128], F8, kind="ExternalInput")
    wlbhi_d = nc.dram_tensor("wlbhi", [80, 2, 112], F8, kind="ExternalInput")
    walo_d = nc.dram_tensor("walo", [128, KS, 2, 2, 128], F8,
                            kind="ExternalInput")
    wblo_d = nc.dram_tensor("wblo", [128, KS, 2, 2, 112], F8,
                            kind="ExternalInput")
    gat_d = nc.dram_tensor("gat", [120, 3, 108], F16, kind="ExternalInput")
    wnd_d = nc.dram_tensor("wnd", [108, 4, 16], F16, kind="ExternalInput")
    wnd8_d = nc.dram_tensor("wnd8", [108, 4, 2, 16], F8,
                            kind="ExternalInput")
    bias_d = nc.dram_tensor("bias", [16, 1], F32, kind="ExternalInput")
    y_d = nc.dram_tensor("y", [16, SLAB, D_OUT, D_OUT], F32,
                         kind="ExternalOutput")

    chunks = [(xr, cy * YB, YB) for xr in range(SLAB) for cy in range(3)]
    chunks = chunks[:-1] + [(SLAB - 1, 24, 8), (SLAB - 1, 32, 4)]

    with TileContext(nc) as tc:
        with tc.tile_pool(name="const", bufs=1) as cpool, \
             tc.tile_pool(name="ca", bufs=2) as capool, \
             tc.tile_pool(name="rr", bufs=4) as rpool, \
             tc.tile_pool(name="fin", bufs=3) as wpool, \
             tc.tile_pool(name="a_ps", bufs=1, space="PSUM") as a_pool, \
             tc.tile_pool(name="b_ps", bufs=1, space="PSUM") as b_pool, \
             tc.tile_pool(name="g_ps", bufs=1, space="PSUM") as g_pool, \
             tc.tile_pool(name="nd_ps", bufs=3, space="PSUM") as nd_pool:

            # ---- resident tiles. DMA order: chunk-0 deps first.
            rallt = cpool.tile([128, 5, SLAB, 2, D_IN, D_OUT], F8,
                               tag="rallt")
            rlt = cpool.tile([80, SLAB, 2, D_OUT, D_OUT], F8, tag="rlt")
            wahit = cpool.tile([128, KS, 3, 2, 128], F8, tag="wahit")
            wbhit = cpool.tile([128, KS, 3, 2, 112], F8, tag="wbhit")
            wlahit = cpool.tile([80, 2, 128], F8, tag="wlahit")
            wlbhit = cpool.tile([80, 2, 112], F8, tag="wlbhit")
            walot = cpool.tile([128, KS, 2, 2, 128], F8, tag="walot")
            wblot = cpool.tile([128, KS, 2, 2, 112], F8, tag="wblot")
            gatt = cpool.tile([120, 3, 108], F16, tag="gatt")
            wndt = cpool.tile([108, 4, 16], F16, tag="wndt")
            wnd8t = cpool.tile([108, 4, 2, 16], F8, tag="wnd8t")
            biast = cpool.tile([16, 1], F32, tag="biast")
            epst = cpool.tile([16, 1], F32, tag="epst")
            nc.vector.memset(epst[:], EPS)

            # chunk-0 critical prefix in first-use order: the first conv
            # matmul (j0,g0) needs only the g0 sliver + wahi; later instrs'
            # deps stream in behind while the PE marches
            nc.sync.dma_start(out=rallt[:, 0, 0, :, 0:17],
                              in_=rall_d[:, 0, 0, :, 0:17])
            nc.sync.dma_start(out=wahit[:], in_=wahi_d[:])
            for gi in range(1, 3):
                nc.sync.dma_start(out=rallt[:, gi, 0, :, 0:17],
                                  in_=rall_d[:, gi, 0, :, 0:17])
            for gi in range(3, 5):
                nc.sync.dma_start(out=rallt[:, gi, 0, :, 0:17],
                                  in_=rall_d[:, gi, 0, :, 0:17])
            nc.sync.dma_start(out=walot[:], in_=walo_d[:])
            nc.sync.dma_start(out=rlt[:, 0, :, 0:13],
                              in_=rl_d[:, 0, :, 0:13])
            nc.sync.dma_start(out=wlahit[:], in_=wlahi_d[:])
            nc.sync.dma_start(out=wbhit[:], in_=wbhi_d[:])
            nc.sync.dma_start(out=wlbhit[:], in_=wlbhi_d[:])
            nc.sync.dma_start(out=wblot[:], in_=wblo_d[:])
            for gi in range(5):
                nc.sync.dma_start(out=rallt[:, gi, 0, :, 17:D_IN],
                                  in_=rall_d[:, gi, 0, :, 17:D_IN])
            nc.sync.dma_start(out=rlt[:, 0, :, 13:D_OUT],
                              in_=rl_d[:, 0, :, 13:D_OUT])
            nc.sync.dma_start(out=gatt[:], in_=gat_d[:])
            nc.sync.dma_start(out=rallt[:, :, 1], in_=rall_d[:, :, 1])
            nc.sync.dma_start(out=rlt[:, 1], in_=rl_d[:, 1])
            nc.sync.dma_start(out=wndt[:], in_=wnd_d[:])
            nc.sync.dma_start(out=wnd8t[:], in_=wnd8_d[:])
            nc.sync.dma_start(out=biast[:], in_=bias_d[:])
            for xr in range(2, SLAB):
                nc.sync.dma_start(out=rallt[:, :, xr], in_=rall_d[:, :, xr])
                nc.sync.dma_start(out=rlt[:, xr], in_=rl_d[:, xr])

            pending = None
            for (xr, y0, yb) in chunks:
                n = yb * D_OUT

                # ---- conv A (120 ch) and conv B-direct (108 ch) ----
                aps = a_pool.tile([120, NCH], F32, tag="aps")
                bps = b_pool.tile([108, NCH], F32, tag="bps")
                for out_ps, m, whi, wlhi, wlo in (
                        (aps, 120, wahit, wlahit, walot),
                        (bps, 108, wbhit, wlbhit, wblot)):
                    t = 0
                    for j in range(KS):
                        for g in range(3):
                            rhs = rallt[:, g, xr, 0:2,
                                        y0 + j:y0 + j + yb, :]
                            nc.tensor.matmul(out_ps[:, 0:n],
                                             whi[:, j, g, :, 0:m],
                                             rhs, start=(t == 0), stop=False,
                                             perf_mode=DR)
                            t += 1
                        for t2 in range(2):
                            rhs = rallt[:, 3 + t2, xr, 0:2,
                                        y0 + j:y0 + j + yb, :]
                            nc.tensor.matmul(out_ps[:, 0:n],
                                             wlo[:, j, t2, :, 0:m],
                                             rhs, start=False, stop=False,
                                             perf_mode=DR)
                    nc.tensor.matmul(out_ps[:, 0:n], wlhi[:, :, 0:m],
                                     rlt[:, xr, 0:2, y0:y0 + yb, :],
                                     start=False, stop=True, perf_mode=DR)

                # conv-A psum -> SBUF fp16 for the grid stage
                ca = capool.tile([120, NCH], F16, tag="ca")
                nc.scalar.copy(ca[:, 0:n], aps[:, 0:n])

                # previous chunk's moment/finalize work: emitted here so the
                # PE never waits on the relu/square of the current chunk
                if pending is not None:
                    _emit_moments(nc, wndt, wnd8t, biast, epst, wpool,
                                  nd_pool, y_d, pending, zero_bias)

                # ---- grid stage mt0..2 (3 matmuls, one PSUM bank each) ----
                gps = g_pool.tile([108, 3, 512], F32, tag="gps")
                for mt in range(3):
                    nc.tensor.matmul(gps[:, mt, 0:n], gatt[:, mt, :],
                                     ca[:, 0:n], start=True, stop=True)

                # ---- relu (scalar, fp16 + fp8 copy) + square (vector) ----
                rrel = rpool.tile([108, 4, NCH], F16, tag="rrel")
                nc.scalar.activation(rrel[:, 0:3, 0:n], gps[:, :, 0:n],
                                     mybir.ActivationFunctionType.Relu)
                nc.scalar.activation(rrel[:, 3, 0:n], bps[:, 0:n],
                                     mybir.ActivationFunctionType.Relu,
                                     scale=inv_sB)
                r2 = rpool.tile([108, 4, NCH], F16, tag="r2", bufs=2)
                nc.vector.tensor_mul(r2[:, :, 0:n], rrel[:, :, 0:n],
                                     rrel[:, :, 0:n])
                r2p = rpool.tile([108, 4, 2, NCH], F8, tag="r2p", bufs=2)
                nc.scalar.copy(r2p[:, :, 0, 0:n], r2[:, :, 0:n])
                nc.vector.tensor_sub(r2p[:, :, 1, 0:n], r2[:, :, 0:n],
                                     r2p[:, :, 0, 0:n])
                pending = (rrel, r2p, xr, y0, yb, n)

            if pending is not None:
                _emit_moments(nc, wndt, wnd8t, biast, epst, wpool, nd_pool,
                              y_d, pending, zero_bias)

    nc.finalize()
    return nc


def _emit_moments(nc, wndt, wnd8t, biast, epst, wpool, nd_pool, y_d, st,
                  zero_bias=False):
    """Moment matmuls (num from r2, den from rrel; 4 chained fp16 each,
    packed in one PSUM bank at column positions 0/64) + finalize + store."""
    rrel, r2p, xr, y0, yb, n = st
    nd = nd_pool.tile([128, NCH], F32, tag="nd")
    for mt in range(4):
        nc.tensor.matmul(nd[0:16, 0:n], wnd8t[:, mt, :, :],
                         r2p[:, mt, :, 0:n],
                         start=(mt == 0), stop=(mt == 3), perf_mode=DR,
                         tile_position=(0, 0))
        nc.tensor.matmul(nd[64:80, 0:n], wndt[:, mt, :], rrel[:, mt, 0:n],
                         start=(mt == 0), stop=(mt == 3),
                         tile_position=(0, 64))
    den_sb = wpool.tile([16, NCH], F32, tag="den_sb")
    nc.vector.tensor_scalar_add(den_sb[:, 0:n], nd[64:80, 0:n],
                                epst[:, 0:1])
    recip = wpool.tile([16, NCH], F32, tag="recip")
    nc.vector.reciprocal(recip[:, 0:n], den_sb[:, 0:n])
    out_sb = wpool.tile([16, NCH], F32, tag="out_sb")
    nc.vector.tensor_mul(out_sb[:, 0:n], nd[0:16, 0:n], recip[:, 0:n])
    if not zero_bias:
        nc.vector.tensor_scalar_add(out_sb[:, 0:n], out_sb[:, 0:n],
                                    biast[:, 0:1])
    nc.sync.dma_start(out=y_d[:, xr, y0:y0 + yb, :],
                      in_=out_sb[:, 0:n].rearrange("p (a b) -> p a b", a=yb))


def _synthesize_filter(weight, zeroweight, basis_functions, wig_w, wig_b):
    """Replicate the reference's kernel synthesis in fp32 numpy.

    Returns kern6[l, e, d, i, j, k] of shape (10, 16, 16, 5, 5, 5) where
    (e,l) indexes the 160 conv output channels and (d,i,j,k) the
    contraction."""
    zero_ext = np.concatenate(
        [zeroweight[None, None],
         np.zeros((ORDER ** 2 - 1, 1, CIN, COUT), weight.dtype)], axis=0)
    wfull = np.concatenate([zero_ext, weight], axis=1)       # (4, 10, 16, 16)
    wg = wfull[wig_w]                                        # (10, 10, 16, 16)
    bg = basis_functions[wig_b]                              # (10, 10, 5, 5, 5)
    kern6 = np.einsum("lred,lrijk->ledijk", wg, bg)          # (10,16,16,5,5,5)
    return np.ascontiguousarray(kern6.astype(np.float32))


def _q8_pair(a):
    hi = a.astype(E4M3)
    lo = (a - hi.astype(np.float32)).astype(E4M3)
    return hi, lo


def _pow2_scale(absmax, target=64.0):
    if absmax <= 0:
        return 1.0
    return 2.0 ** np.floor(np.log2(target / absmax))


def _host_prep(x, weight, zeroweight, bias, so3basisgrid, w_i,
               basis_functions, wig_w, wig_b):
    kern6 = _synthesize_filter(weight, zeroweight, basis_functions,
                               wig_w, wig_b)
    # Wf[pair(i,j), d, k, out(e*10+l)]
    Wf = np.ascontiguousarray(
        kern6.transpose(3, 4, 2, 5, 1, 0).reshape(25, 16, KS, 160))

    g2 = np.asarray(so3basisgrid, np.float32).reshape(27, S)  # [mln, l]
    w_flat = np.asarray(w_i, np.float32)[(np.arange(27) // 3) % 3]

    # B channels (e>=12): fold grid -> out (el2*27+mln)
    WfB = Wf[:, :, :, 120:].reshape(25, 16, KS, 4, S)        # (..., el2, l)
    kern2B = np.einsum("pdkel,ml->pdkem", WfB, g2).reshape(25, 16, KS, 108)
    WfA = Wf[:, :, :, 0:120]

    sA = _pow2_scale(np.abs(WfA).max())
    sB = _pow2_scale(np.abs(kern2B).max())
    WfA_hi, WfA_lo = _q8_pair(WfA * sA)          # (25, 16, KS, 120) e4m3
    kB_hi, kB_lo = _q8_pair(kern2B * sB)

    # conv lhsT tiles. Row space: rho = pair2*16+d with pair2 = i*5+k; the
    # j-tap picks the weight slice and the rhs y-offset. M padded to mp for
    # the 16B dual-fp8 pair-stride rule.
    # W2[pair2, d, j, out] = Wf[i*5+j, d, k, out]
    def reindex(w):
        m = w.shape[-1]
        w5 = w.reshape(KS, KS, 16, KS, m)                # (i, j, d, k, m)
        return np.ascontiguousarray(
            w5.transpose(0, 3, 2, 1, 4).reshape(25, 16, KS, m))

    def pack_hi(w_hi, m, mp):
        w_hi = reindex(w_hi)
        out = np.zeros((128, KS, 3, 2, mp), E4M3)
        for pair in range(24):
            g, p0 = divmod(pair * 16, 128)
            out[p0:p0 + 16, :, g, 0, 0:m] = w_hi[pair]   # (16, KS, m)
            out[p0:p0 + 16, :, g, 1, 0:m] = w_hi[pair]
        return out

    def pack_lo(w_lo, m, mp):
        # term2 k-tile pairs: t=0 -> rows (0..127 | 128..255),
        # t=1 -> rows (256..383 | 384..399 zero-padded)
        w_lo = reindex(w_lo)
        out = np.zeros((128, KS, 2, 2, mp), E4M3)
        for pair in range(25):
            rho0 = pair * 16
            t, rem = divmod(rho0, 256)
            gg, p0 = divmod(rem, 128)
            out[p0:p0 + 16, :, t, gg, 0:m] = w_lo[pair]
        return out

    def pack_leftover_hi(w_hi, m, mp):
        # leftover pair2 = (i=4, k=4); rows (j, d)
        w_hi = reindex(w_hi)
        out = np.zeros((80, 2, mp), E4M3)
        for j in range(KS):
            out[16 * j:16 * j + 16, 0, 0:m] = w_hi[24, :, j, :]
            out[16 * j:16 * j + 16, 1, 0:m] = w_hi[24, :, j, :]
        return out

    wahi = pack_hi(WfA_hi, 120, 128)
    wbhi = pack_hi(kB_hi, 108, 112)
    walo = pack_lo(WfA_lo, 120, 128)
    wblo = pack_lo(kB_lo, 108, 112)
    wlahi = pack_leftover_hi(WfA_hi, 120, 128)
    wlbhi = pack_leftover_hi(kB_hi, 108, 112)

    # grid lhsT for mt0..2: rows (e*10+l, e<12); undo sA here
    gat = np.zeros((120, 3, 108), np.float32)
    for mt in range(3):
        for el2 in range(4):
            e = 4 * mt + el2
            for l in range(S):
                gat[e * S + l, mt, el2 * 27:(el2 + 1) * 27] = g2[:, l]
    gat = (gat / sA).astype(np.float16)

    # moment lhsTs: rows (el2*27+mln) -> col e = 4*mt+el2.
    # mt3 rows see sB-scaled relu values: undo with 1/sB^2 (num), 1/sB (den)
    wnd = np.zeros((108, 4, 16), np.float32)
    for mt in range(4):
        for el2 in range(4):
            e = 4 * mt + el2
            wnd[el2 * 27:(el2 + 1) * 27, mt, e] = w_flat
    # quantize w once (opt scale); num uses it as fp8 DR lhsT, den uses
    # the identical values in fp16 so the num/den ratio cancels the scale
    wvals = np.asarray(w_i, np.float32)
    best, s_wnd = 1e9, 64.0
    for s in np.linspace(32.0, 128.0, 2048):
        q = (wvals * s).astype(E4M3).astype(np.float32)
        e = np.abs(q / (wvals * s) - 1).max()
        if e < best:
            best, s_wnd = e, float(s)
    wq8 = (wnd * s_wnd).astype(E4M3)                  # (108, 4, 16)
    wnd8 = np.zeros((108, 4, 2, 16), E4M3)
    wnd8[:, :, 0, :] = wq8
    wnd8[:, :, 1, :] = wq8
    wnd = wq8.astype(np.float16)


    bias_arr = np.asarray(bias, np.float32).reshape(16, 1)

    # input hi/lo quantization (global, then per-core packing)
    x32 = np.asarray(x, np.float32)
    x_hi8 = x32.astype(E4M3)
    x_lo8 = (x32 - x_hi8.astype(np.float32)).astype(E4M3)

    in_maps = []
    for c in range(NCORES):
        b, q = divmod(c, 4)
        sl = slice(q * SLAB, q * SLAB + SLAB_IN)
        shi = x_hi8[b, :, sl]                    # (16, 13, 40, 40) e4m3
        slo = x_lo8[b, :, sl]
        rall = np.zeros((128, 5, SLAB, 2, D_IN, D_OUT), E4M3)
        for pair in range(25):
            i, k = divmod(pair, KS)
            hi_blk = shi[:, i:i + SLAB, :, k:k + D_OUT]      # (16,9,40,36)
            if pair < 24:
                g, p0 = divmod(pair * 16, 128)
                lo_blk = slo[:, i:i + SLAB, :, k:k + D_OUT]
                rall[p0:p0 + 16, g, :, 0] = hi_blk
                rall[p0:p0 + 16, g, :, 1] = lo_blk
            t, rem = divmod(pair * 16, 256)
            gg, p0 = divmod(rem, 128)
            rall[p0:p0 + 16, 3 + t, :, gg] = hi_blk
        rl = np.empty((80, SLAB, 2, D_OUT, D_OUT), E4M3)
        for j in range(KS):
            rl[16 * j:16 * j + 16, :, 0] = shi[:, 4:4 + SLAB, j:j + D_OUT,
                                               4:4 + D_OUT]
            rl[16 * j:16 * j + 16, :, 1] = slo[:, 4:4 + SLAB, j:j + D_OUT,
                                               4:4 + D_OUT]
        in_maps.append({
            "rall": rall, "rl": rl,
            "wahi": wahi, "wbhi": wbhi, "wlahi": wlahi, "wlbhi": wlbhi,
            "walo": walo, "wblo": wblo,
            "gat": np.ascontiguousarray(gat),
            "wnd": np.ascontiguousarray(wnd),
            "wnd8": np.ascontiguousarray(wnd8),
            "bias": bias_arr,
        })
    return in_maps, sB


def _run(inputs, trace=False, **run_kwargs):
    inputs = {k: np.asarray(v) for k, v in inputs.items()}
    in_maps, sB = _host_prep(**inputs)
    zb = bool(np.all(np.asarray(inputs["bias"]) == 0.0))
    if _prog_cache.get("key") != (float(sB), zb):
        _prog_cache["nc"] = _build_program(1.0 / sB, zb)
        _prog_cache["key"] = (float(sB), zb)
    nc = _prog_cache["nc"]
    try:
        res = run_bass_kernel_spmd(nc, in_maps, core_ids=list(range(NCORES)),
                                   trace=trace, **run_kwargs)
    except ModuleNotFoundError as e:
        if "axon_hooks" not in str(e):
            raise
        os.environ["BASS_NEVER_TRACE"] = "1"
        res = run_bass_kernel_spmd(nc, in_maps, core_ids=list(range(NCORES)),
                                   trace=False, **run_kwargs)
    out = np.empty((B, COUT, D_OUT, D_OUT, D_OUT), np.float32)
    for c in range(NCORES):
        b, q = divmod(c, 4)
        out[b, :, q * SLAB:(q + 1) * SLAB] = res.results[c]["y"]
    return out, res


def kernel(**inputs):
    out, _ = _run(inputs)
    return out
